# revision 82
# baseline (speedup 1.0000x reference)
"""Trainium2 Bass kernel for EnhancedAttention (B=2, T=2048, D=1024, H=16, DH=64).

Sharding: 8 cores = 2 batches x 4 head-groups (4 heads each). No collectives;
each core computes a partial out-projection (bf16) and the host sums the 4
partials per batch in f32.

v4: head-PAIR attention units with row-tiled S matmuls (the K=64 QK^T
matmuls for the two heads of a pair run concurrently on PE row-groups
0-63 / 64-127), chunk order 0->3 so the densest chunk drains last (keeps
the PE clock warm), host-side pre-arranged input layouts for 4-8KB DMA
descriptors, rope rotate-half as one matmul against a constant
permutation (no cross-partition DMAs), softmax denominator replicated
into PSUM partitions 64-127 by a 64-wide ones block in the V operand
(normalize = reciprocal + multiply, no broadcast matmul / den DMA),
all causal masks on DVE, a flat S-slot stream with a fixed-lag AV
zipper, and bf16 y output spread across the sync/gpsimd DMA rings.
"""
import os
import sys

for _p in ("/opt/trn_rl_repo", "/root/.axon_site/_ro/trn_rl_repo"):
    if os.path.isdir(_p) and _p not in sys.path:
        sys.path.append(_p)

import ml_dtypes
import numpy as np

import concourse.bass as bass  # noqa: F401
import concourse.tile as tile
from concourse import bacc, mybir
from concourse.bass_utils import run_bass_kernel_spmd

B, T, D = 2, 2048, 1024
H, DH = 16, 64
HPC = 4  # heads per core
NCORES = 8
ROPE_THETA = 10000.0

F32 = mybir.dt.float32
BF16 = mybir.dt.bfloat16

TCH = 512  # t-chunk (q-chunk) size
TC = T // TCH  # 4
DC = D // 128  # 8 contraction chunks
NKT = T // 128  # 16 k-tiles

def _rope_tables():
    """Full 128-partition rope tables (inlined; avoids SBUF broadcasts).

    cs2 is PRE-SWAPPED: the kernel block-swaps the raw projection (via the
    constant permutation matmul) instead of the cs2 product, so the sign
    pattern that normally sits on the swapped product moves into the table:
    row p holds cs2_orig[swap32(p)] = [-sin, sin, -sin, sin] blocks.
    """
    inv = 1.0 / (ROPE_THETA ** (np.arange(0, DH, 2, dtype=np.float64) / DH))
    f = np.arange(T, dtype=np.float64)[:, None] * inv[None, :]  # [T, 32]
    cos = np.cos(f).T.astype(ml_dtypes.bfloat16)  # [32, T]
    sin = np.sin(f).T.astype(ml_dtypes.bfloat16)
    cs1 = np.ascontiguousarray(np.tile(cos, (4, 1)))  # [128, T]
    cs2s = np.ascontiguousarray(
        np.concatenate([-sin, sin, -sin, sin], axis=0)
    )  # [128, T]
    return cs1, cs2s


def _build():
    nc = bacc.Bacc("TRN2", target_bir_lowering=False, debug=False, num_devices=NCORES)
    # host pre-arranged layouts (see kernel() below):
    #   xp:  [TC*128, DC*TCH]  chunk tci rows tci*128..: [p][dc*TCH+t']
    #   wq/wk/wv: [128, DC*HPC*DH]   [p][dc*256+n]
    #   wo:  [128, 2*D]              [p][p2*D+n]
    xp_d = nc.dram_tensor("xp", [TC * 128, DC * TCH], BF16, kind="ExternalInput")
    # wq/wk stored as [128, 2(p-half), DC, 128] so each half can be DMA'd
    # separately (the first attention unit only needs half p=0)
    wq_d = nc.dram_tensor("wq", [128, 2 * DC * 128], BF16, kind="ExternalInput")
    wk_d = nc.dram_tensor("wk", [128, 2 * DC * 128], BF16, kind="ExternalInput")
    wv_d = nc.dram_tensor("wv", [128, DC * HPC * DH], BF16, kind="ExternalInput")
    wo_d = nc.dram_tensor("wo", [128, 2 * D], BF16, kind="ExternalInput")
    y_d = nc.dram_tensor("y", [T, D], BF16, kind="ExternalOutput")

    cs1_np, cs2_np = _rope_tables()
    cs1_d = nc.inline_tensor(cs1_np, "cs1")  # [128, T]
    cs2_d = nc.inline_tensor(cs2_np, "cs2")  # [128, T]

    # causal masks (keep = 1.0): maskt for the kt0 diagonal 128-block,
    # maskd = [zeros(128) | tri(128)] for the extended kt1 diagonal block
    maskt_np = (np.arange(128)[None, :] >= np.arange(128)[:, None])
    maskd_np = (np.arange(256)[None, :] >= np.arange(128)[:, None] + 128)
    maskt_d = nc.inline_tensor(
        np.ascontiguousarray(maskt_np.astype(ml_dtypes.bfloat16)), "maskt"
    )
    maskd_d = nc.inline_tensor(
        np.ascontiguousarray(maskd_np.astype(ml_dtypes.bfloat16)), "maskd"
    )
    # 32-block-swap permutation matrix (symmetric): rope's rotate-half is
    # done as one extra matmul instead of cross-partition DMAs
    perm_np = np.zeros((128, 128), dtype=np.float32)
    sigma = (np.arange(128) + 32) % 64 + (np.arange(128) // 64) * 64
    perm_np[np.arange(128), sigma] = 1.0
    perm_d = nc.inline_tensor(
        np.ascontiguousarray(perm_np.astype(ml_dtypes.bfloat16)), "perm"
    )
    EXP = mybir.ActivationFunctionType.Exp
    COPYF = mybir.ActivationFunctionType.Copy

    import contextlib
    with tile.TileContext(nc) as tc:
        with (
            contextlib.ExitStack() as _ctx,
            tc.tile_pool(name="sb", bufs=1) as sb,
            tc.tile_pool(name="ropep", bufs=2) as ropep,
            tc.tile_pool(name="ptp", bufs=6) as ptp,
            tc.tile_pool(name="recp", bufs=2) as recp,
            tc.tile_pool(name="ysbp", bufs=3) as ysbp,
        ):
            wq = sb.tile([128, 2, DC, 128], BF16)
            wk = sb.tile([128, 2, DC, 128], BF16)
            wv = sb.tile([128, DC, HPC * DH], BF16)
            wo = sb.tile([128, 2, D], BF16)
            cs1 = sb.tile([128, T], BF16)
            cs2 = sb.tile([128, T], BF16)
            qt = [sb.tile([128, T], BF16, tag=f"qt{p}", name=f"qt{p}") for p in range(2)]
            ktt = [sb.tile([128, T], BF16, tag=f"kt{p}", name=f"kt{p}") for p in range(2)]
            maskt = sb.tile([128, 128], BF16, name="maskt")
            maskd = sb.tile([128, 256], BF16, name="maskd")
            perm = sb.tile([128, 128], BF16, name="perm")
            # v augmented with a 64-wide block of ones: the AV matmul then
            # replicates the softmax denominator across PSUM partitions
            # 64..127, so normalize needs no broadcast matmul or den DMA
            vaug = sb.tile([128, NKT // 2, 2, HPC, 2 * DH], BF16, name="vaug")
            ot = [sb.tile([128, T], BF16, tag=f"ot{p}", name=f"ot{p}") for p in range(2)]
            xts = [
                sb.tile([128, DC, TCH], BF16, tag=f"xt{tci}", name=f"xt{tci}")
                for tci in range(TC)
            ]

            # ---------------- startup DMAs ----------------
            # DMA rings stall their engine after ~6 outstanding issues (ring
            # credits), so each ring gets at most ~5 upfront; the rest are
            # issued from filler thunks once earlier transfers have drained.
            # Needed-first order: chunk-0 x split across sync+gpsimd, qk
            # weight halves on scalar, rope tables on gpsimd.
            xp_r = xp_d.ap().rearrange("(c p) (d t) -> p c d t", p=128, d=DC)
            wq_r = wq_d.ap().rearrange("p (h c n) -> p h c n", h=2, c=DC)
            wk_r = wk_d.ap().rearrange("p (h c n) -> p h c n", h=2, c=DC)
            # chunk-0 x in 2-dc pieces so the first proj matmul's dc
            # accumulation can begin as soon as the first 256KB lands
            nc.sync.dma_start(xts[0][:, 0:2], xp_r[:, 0, 0:2])
            nc.sync.dma_start(xts[0][:, 2:4], xp_r[:, 0, 2:4])
            nc.sync.dma_start(perm[:], perm_d.ap())
            nc.sync.dma_start(maskt[:], maskt_d.ap())
            nc.sync.dma_start(maskd[:], maskd_d.ap())
            nc.sync.dma_start(xts[1][:], xp_r[:, 1])
            nc.scalar.dma_start(wq[:, 0], wq_r[:, 0])
            nc.scalar.dma_start(wk[:, 0], wk_r[:, 0])
            nc.scalar.dma_start(wq[:, 1], wq_r[:, 1])
            nc.scalar.dma_start(wk[:, 1], wk_r[:, 1])
            nc.scalar.dma_start(wv[:], wv_d.ap().rearrange("p (c n) -> p c n", c=DC))
            nc.gpsimd.dma_start(xts[0][:, 4:6], xp_r[:, 0, 4:6])
            nc.gpsimd.dma_start(xts[0][:, 6:8], xp_r[:, 0, 6:8])
            nc.gpsimd.dma_start(cs1[:], cs1_d.ap())
            nc.gpsimd.dma_start(cs2[:], cs2_d.ap())

            def load_xp2_wo():
                nc.scalar.dma_start(xts[2][:], xp_r[:, 2])
                nc.scalar.dma_start(wo[:], wo_d.ap().rearrange("p (a n) -> p a n", a=2))

            def load_xp3():
                nc.scalar.dma_start(xts[3][:], xp_r[:, 3])

            # big strided memset rides the otherwise-idle gpsimd engine so the
            # DVE (whose first op gates the PE warmup) stays clear
            nc.gpsimd.memset(vaug[:, :, :, :, DH:], 1.0)

            # PSUM: pjps(2) + sps(2 tags x 2 banks) + ops(2 tags x 1) = 8 banks
            pjps = _ctx.enter_context(tc.tile_pool(name="pjps", bufs=2, space="PSUM"))
            sps = _ctx.enter_context(tc.tile_pool(name="sps", bufs=1, space="PSUM"))
            ops = _ctx.enter_context(tc.tile_pool(name="ops", bufs=1, space="PSUM"))

            # PE warm-up: covers the startup DMA latency; ramps the PE p-state
            warm = sb.tile([128, TCH], BF16, name="warm")
            nc.vector.memset(warm, 0.0)
            wps = pjps.tile([128, TCH], F32, tag="pj", name="wps")
            for wi in range(9):
                nc.tensor.matmul(
                    wps[:], warm[:, 0:128], warm[:],
                    start=(wi == 0), stop=(wi == 8),
                )

            # ---------------- projection pieces ----------------
            def gen_proj_pieces(tci):
                """8 pieces: [q p0, q p1, k p0, k p1, v0..v3] (each ~8 MMs)."""
                tsl = slice(tci * TCH, (tci + 1) * TCH)
                xt = xts[tci]

                def qk_piece(w_sb, dest, p, pi):
                    is_q = dest is qt

                    def run():
                        ps = pjps.tile([128, TCH], F32, tag="pj", name=f"pj{tci}_{pi}")
                        for dc in range(DC):
                            nc.tensor.matmul(
                                ps[:],
                                w_sb[:, p, dc, :],
                                xt[:, dc, :],
                                start=(dc == 0),
                                stop=(dc == DC - 1),
                            )
                        # rope: scalar evacuates the psum to bf16; the
                        # 32-block swap is one matmul against the constant
                        # permutation (cs2 table pre-swapped to match), so the
                        # rope chain has no cross-partition DMAs at all
                        qkbf = ropep.tile([128, TCH], BF16, tag="qkbf", name=f"qb{tci}_{pi}")
                        nc.scalar.activation(qkbf[:], ps[:], COPYF, bias=0.0, scale=1.0)
                        ps2 = pjps.tile([128, TCH], F32, tag="pj", name=f"pw{tci}_{pi}")
                        nc.tensor.matmul(ps2[:], perm[:], qkbf[:], start=True, stop=True)
                        t1 = ropep.tile([128, TCH], BF16, tag="t1", name=f"t1_{tci}_{pi}")
                        t2 = ropep.tile([128, TCH], BF16, tag="t2", name=f"t2_{tci}_{pi}")
                        nc.vector.tensor_mul(t1[:], qkbf[:], cs1[:, tsl])
                        nc.vector.tensor_mul(t2[:], ps2[:], cs2[:, tsl])
                        nc.vector.tensor_add(dest[p][:, tsl], t1[:], t2[:])
                    return run

                def v_piece(tt):
                    def run():
                        gt = tci * 4 + tt
                        ps = pjps.tile([128, TCH], F32, tag="pj", name=f"pjv{gt}")
                        for dc in range(DC):
                            nc.tensor.matmul(
                                ps[:, : HPC * DH],
                                xt[:, dc, tt * 128 : (tt + 1) * 128],
                                wv[:, dc, :],
                                start=(dc == 0),
                                stop=(dc == DC - 1),
                            )
                        nc.vector.tensor_copy(
                            vaug[:, gt // 2, gt % 2, :, 0:DH],
                            ps[:, : HPC * DH].rearrange("p (h d) -> p h d", h=HPC),
                        )
                    return run

                pieces = []
                pi = 0
                for w_sb, dest in ((wq, qt), (wk, ktt)):
                    for p in range(2):
                        pieces.append(qk_piece(w_sb, dest, p, pi))
                        pi += 1
                for tt in range(4):
                    pieces.append(v_piece(tt))
                return pieces

            # ---------------- out-projection pieces ----------------
            YRINGS = [nc.sync, nc.gpsimd, nc.sync, nc.gpsimd]

            def gen_outproj_pieces(qc):
                def piece(tt):
                    def run():
                        gtt = qc * 4 + tt
                        ysb = ysbp.tile([128, 2, TCH], BF16, tag="ysb", name=f"ys{gtt}")
                        for ni in range(2):
                            ypsum = pjps.tile(
                                [128, TCH], F32, tag="pj", name=f"y{gtt}_{ni}"
                            )
                            for p2 in range(2):
                                nc.tensor.matmul(
                                    ypsum[:],
                                    ot[p2][:, gtt * 128 : (gtt + 1) * 128],
                                    wo[:, p2, ni * TCH : (ni + 1) * TCH],
                                    start=(p2 == 0),
                                    stop=(p2 == 1),
                                )
                            nc.vector.tensor_copy(ysb[:, ni, :], ypsum[:])
                        YRINGS[tt].dma_start(
                            y_d.ap()[gtt * 128 : (gtt + 1) * 128, :],
                            ysb.rearrange("p a b -> p (a b)"),
                        )
                    return run
                return [piece(tt) for tt in range(4)]

            # ---------------- attention: head-pair units ----------------
            pts = {}     # (qc,hp) -> {g: [pt_par0, pt_par1]}
            opsums = {}  # (qc,hp) -> [psum_par0, psum_par1]

            def emit_s_group(qc, hp, g):
                """Row-tiled S pair + exp + mask for k-tile pair (2g, 2g+1)."""
                kt0 = 2 * g
                off0 = max(0, 128 * kt0 - TCH * qc)
                qsl = slice(qc * TCH, (qc + 1) * TCH)
                spts, ptts = [], []
                for par in (0, 1):
                    spt = sps.tile(
                        [128, 2, TCH], F32, tag=f"s{par}", name=f"s{qc}_{hp}_{g}_{par}"
                    )
                    pt = ptp.tile(
                        [128, 2, TCH], BF16, tag=f"pt{par}", name=f"p{qc}_{hp}_{g}_{par}"
                    )
                    spts.append(spt)
                    ptts.append(pt)
                pts[(qc, hp)][g] = ptts
                for j in (0, 1):
                    kt = kt0 + j
                    for par in (0, 1):
                        nc.tensor.matmul(
                            spts[par][:, j, off0:],
                            ktt[hp][par * 64 : (par + 1) * 64, kt * 128 : (kt + 1) * 128],
                            qt[hp][par * 64 : (par + 1) * 64, qsl][:, off0:],
                            start=True,
                            stop=True,
                        )
                for par in (0, 1):
                    if off0 == 0:
                        nc.scalar.activation(
                            ptts[par].rearrange("p a b -> p (a b)"),
                            spts[par].rearrange("p a b -> p (a b)"),
                            EXP, bias=0.0, scale=0.125,
                        )
                    else:
                        nc.scalar.activation(
                            ptts[par][:, :, off0:], spts[par][:, :, off0:],
                            EXP, bias=0.0, scale=0.125,
                        )
                if kt0 >= 4 * qc:  # diagonal pair
                    for par in (0, 1):
                        nc.vector.tensor_mul(
                            ptts[par][:, 0, off0 : off0 + 128],
                            ptts[par][:, 0, off0 : off0 + 128],
                            maskt[:],
                        )
                        nc.vector.tensor_mul(
                            ptts[par][:, 1, off0 : off0 + 256],
                            ptts[par][:, 1, off0 : off0 + 256],
                            maskd[:],
                        )

            def emit_av_group(qc, hp, g):
                nkt = 4 * qc + 4
                kt0 = 2 * g
                ptts = pts[(qc, hp)].pop(g)
                if g == 0:
                    opsums[(qc, hp)] = ops.tile(
                        [128, 2, TCH], F32, tag="o", name=f"o{qc}_{hp}"
                    )
                for par in (0, 1):
                    h = 2 * hp + par
                    for j in (0, 1):
                        kt = kt0 + j
                        off = max(0, 128 * kt - TCH * qc)
                        nc.tensor.matmul(
                            opsums[(qc, hp)][:, par, off:],
                            vaug[:, g, j, h, :],
                            ptts[par][:, j, off:],
                            start=(kt == 0),
                            stop=(kt == nkt - 1),
                        )

            def emit_evac_norm(qc, hp):
                """Normalize both heads straight out of PSUM: partitions
                64-127 of each AV psum hold the softmax denominator (vaug's
                ones block), so a reciprocal + one multiply per head does it.
                ot layout par-swapped: partitions 0-63 = odd head, 64-127 =
                even head."""
                qsl = slice(qc * TCH, (qc + 1) * TCH)
                opsum = opsums[(qc, hp)]
                den = recp.tile([64, 2, TCH], F32, tag="dn", name=f"dn{qc}_{hp}")
                # custom DVE ops can't source PSUM: bounce den via SBUF
                nc.vector.tensor_copy(den[:], opsum[DH : 2 * DH, :, :])
                rec = recp.tile([64, 2, TCH], F32, tag="rc", name=f"rc{qc}_{hp}")
                nc.vector.reciprocal_approx_fast(
                    out=rec.rearrange("p a b -> p (a b)"),
                    in_=den.rearrange("p a b -> p (a b)"),
                )
                nc.vector.tensor_mul(
                    ot[hp][0:64, qsl], opsum[0:DH, 1, :], rec[:, 1, :]
                )
                nc.vector.tensor_mul(
                    ot[hp][64:128, qsl], opsum[0:DH, 0, :], rec[:, 0, :]
                )

            # ---------------- emission schedule ----------------
            # proj(0) upfront; the two pieces the first attention unit needs
            # (q p0, k p0) go first
            p0 = gen_proj_pieces(0)
            for f in [p0[0], p0[2], p0[1], p0[3]] + p0[4:]:
                f()

            p1 = gen_proj_pieces(1)
            p2 = gen_proj_pieces(2)
            p3 = gen_proj_pieces(3)

            op0 = gen_outproj_pieces(0)
            op1 = gen_outproj_pieces(1)
            op2 = gen_outproj_pieces(2)

            units = [(qc, hp) for qc in range(TC) for hp in range(2)]
            # fillers per unit (consumed one per S-group slot; leftovers run
            # at the unit boundary). Invocation order must respect the norms
            # each outproj chunk reads.
            unit_fillers = {
                0: [load_xp2_wo, load_xp3],
                1: [p1[0], p1[2], p1[1], p1[3]],      # q1/k1 (2 slots + spill)
                2: p1[4:8],                           # v(1)
                3: [p2[0], p2[2], p2[1], p2[3]] + op0,
                4: p2[4:8] + [p3[0], p3[2]],
                5: [p3[1], p3[3]] + op1,
                6: p3[4:8],                           # v(3)
                7: op2[0:2],  # op2[2:4] held back to cover the drain norm
            }

            # flat S-slot stream with a fixed AV lag: the AV matmuls for
            # S slot i run at slot i+LAG, so the final unit's AV doesn't
            # bunch up in the drain
            LAG = 4
            slots = []
            for ui, (qc, hp) in enumerate(units):
                for g in range((4 * qc + 4) // 2):
                    slots.append((ui, qc, hp, g))

            def do_av_slot(j):
                ui, qc, hp, g = slots[j]
                emit_av_group(qc, hp, g)
                if g == (4 * qc + 4) // 2 - 1:  # unit's last AV group
                    emit_evac_norm(qc, hp)

            fill = []
            for i, (ui, qc, hp, g) in enumerate(slots):
                if g == 0:
                    fill = list(unit_fillers.get(ui, []))
                    pts[(qc, hp)] = {}
                    opsums[(qc, hp)] = None
                if fill:
                    fill.pop(0)()
                if i >= LAG:
                    do_av_slot(i - LAG)
                emit_s_group(qc, hp, g)
                if g == (4 * qc + 4) // 2 - 1:
                    for f in fill:
                        f()
                    fill = []

            # drain: the last LAG AV slots, final norm, outproj of last chunk.
            # The outproj is emitted ni-granular with both p2=0 matmuls (which
            # depend only on norm(3,0)) ahead of the p2=1 ones, so the PE has
            # work while the final norm's DVE chain runs; casts ride the
            # now-idle scalar engine.
            for j in range(len(slots) - LAG, len(slots)):
                do_av_slot(j)
            op2[2]()
            op2[3]()
            # last chunk's outproj: the freed S psum banks hold 2-bank ypsums,
            # p2=0 matmuls (independent of the final norm) lead, one merged
            # scalar cast per piece
            for tt in range(4):
                gtt = 3 * 4 + tt
                ypsum = sps.tile(
                    [128, 2, TCH], F32, tag=f"s{tt % 2}", name=f"y{gtt}"
                )
                for p2 in range(2):
                    for ni in range(2):
                        nc.tensor.matmul(
                            ypsum[:, ni, :],
                            ot[p2][:, gtt * 128 : (gtt + 1) * 128],
                            wo[:, p2, ni * TCH : (ni + 1) * TCH],
                            start=(p2 == 0), stop=(p2 == 1),
                        )
                ysb = ysbp.tile([128, 2, TCH], BF16, tag="ysb", name=f"ys{gtt}")
                nc.scalar.activation(
                    ysb.rearrange("p a b -> p (a b)"),
                    ypsum.rearrange("p a b -> p (a b)"),
                    COPYF, bias=0.0, scale=1.0,
                )
                YRINGS[tt].dma_start(
                    y_d.ap()[gtt * 128 : (gtt + 1) * 128, :],
                    ysb.rearrange("p a b -> p (a b)"),
                )
    nc.compile()
    return nc


_NC_CACHE = []


def _get_nc():
    if not _NC_CACHE:
        _NC_CACHE.append(_build())
    return _NC_CACHE[0]


_LAST_RESULTS = []  # stashed BassKernelResults for test harness introspection


def _wo_rows_parswap(Wout_rows):
    """Reorder the 256 Wout rows so each 128-row pair block is [odd-head 64 | even-head 64]."""
    out = np.empty_like(Wout_rows)
    for hp in range(2):
        blk = Wout_rows[hp * 128 : (hp + 1) * 128]
        out[hp * 128 : hp * 128 + 64] = blk[64:128]
        out[hp * 128 + 64 : (hp + 1) * 128] = blk[0:64]
    return out


def kernel(x, Wqkv, Wout, _trace=False, **_trace_kwargs):
    x = np.asarray(x, dtype=np.float32)
    Wqkv = np.asarray(Wqkv, dtype=np.float32)
    Wout = np.asarray(Wout, dtype=np.float32)

    nc = _get_nc()
    bf = ml_dtypes.bfloat16
    in_maps = []
    for c in range(NCORES):
        b, g = divmod(c, HPC)
        cols = slice(g * HPC * DH, (g + 1) * HPC * DH)
        rows = slice(g * HPC * DH, (g + 1) * HPC * DH)
        xT = x[b].T.astype(bf)  # [D, T]
        xp = np.ascontiguousarray(
            xT.reshape(DC, 128, TC, TCH).transpose(2, 1, 0, 3).reshape(TC * 128, DC * TCH)
        )

        def wprep(w):  # [D, 256] -> [128, DC*256]
            return np.ascontiguousarray(
                w.astype(bf).reshape(DC, 128, HPC * DH).transpose(1, 0, 2).reshape(128, -1)
            )

        def wprep_qk(w):  # [D, 256] -> [128, 2*DC*128] ([p][half][dc][n])
            return np.ascontiguousarray(
                w.astype(bf).reshape(DC, 128, 2, 128).transpose(1, 2, 0, 3).reshape(128, -1)
            )

        wo_ = np.ascontiguousarray(
            _wo_rows_parswap(Wout[rows, :]).astype(bf)
            .reshape(2, 128, D).transpose(1, 0, 2).reshape(128, 2 * D)
        )
        in_maps.append(
            {
                "xp": xp,
                "wq": wprep_qk(Wqkv[:, 0:D][:, cols]),
                "wk": wprep_qk(Wqkv[:, D : 2 * D][:, cols]),
                "wv": wprep(Wqkv[:, 2 * D : 3 * D][:, cols]),
                "wo": wo_,
            }
        )

    res = run_bass_kernel_spmd(
        nc, in_maps, core_ids=list(range(NCORES)), trace=_trace, **_trace_kwargs
    )
    _LAST_RESULTS.clear()
    _LAST_RESULTS.append(res)

    out = np.zeros((B, T, D), dtype=np.float32)
    for c in range(NCORES):
        b = c // HPC
        out[b] += res.results[c]["y"].astype(np.float32)
    return out


# revision 84
# speedup vs baseline: 1.1986x; 1.1986x over previous
"""Trainium2 Bass kernel for EnhancedAttention (B=2, T=2048, D=1024, H=16, DH=64).

Sharding: 8 cores = 2 batches x 4 head-groups (4 heads each). No collectives;
each core computes a partial out-projection (bf16) and the host sums the 4
partials per batch in f32.

v4: head-PAIR attention units with row-tiled S matmuls (the K=64 QK^T
matmuls for the two heads of a pair run concurrently on PE row-groups
0-63 / 64-127), chunk order 0->3 so the densest chunk drains last (keeps
the PE clock warm), host-side pre-arranged input layouts for 4-8KB DMA
descriptors, rope rotate-half as one matmul against a constant
permutation (no cross-partition DMAs), softmax denominator replicated
into PSUM partitions 64-127 by a 64-wide ones block in the V operand
(normalize = reciprocal + multiply, no broadcast matmul / den DMA),
all causal masks on DVE, a flat S-slot stream with a fixed-lag AV
zipper, and bf16 y output spread across the sync/gpsimd DMA rings.
"""
import os
import sys

for _p in ("/opt/trn_rl_repo", "/root/.axon_site/_ro/trn_rl_repo"):
    if os.path.isdir(_p) and _p not in sys.path:
        sys.path.append(_p)

import ml_dtypes
import numpy as np

import concourse.bass as bass  # noqa: F401
import concourse.tile as tile
from concourse import bacc, mybir
from concourse.bass_utils import run_bass_kernel_spmd

B, T, D = 2, 2048, 1024
H, DH = 16, 64
HPC = 4  # heads per core
NCORES = 8
ROPE_THETA = 10000.0

F32 = mybir.dt.float32
BF16 = mybir.dt.bfloat16

TCH = 512  # t-chunk (q-chunk) size
TC = T // TCH  # 4
DC = D // 128  # 8 contraction chunks
NKT = T // 128  # 16 k-tiles

def _rope_tables():
    """Full 128-partition rope tables (inlined; avoids SBUF broadcasts).

    cs2 is PRE-SWAPPED: the kernel block-swaps the raw projection (via the
    constant permutation matmul) instead of the cs2 product, so the sign
    pattern that normally sits on the swapped product moves into the table:
    row p holds cs2_orig[swap32(p)] = [-sin, sin, -sin, sin] blocks.
    """
    inv = 1.0 / (ROPE_THETA ** (np.arange(0, DH, 2, dtype=np.float64) / DH))
    f = np.arange(T, dtype=np.float64)[:, None] * inv[None, :]  # [T, 32]
    cos = np.cos(f).T.astype(ml_dtypes.bfloat16)  # [32, T]
    sin = np.sin(f).T.astype(ml_dtypes.bfloat16)
    cs1 = np.ascontiguousarray(np.tile(cos, (4, 1)))  # [128, T]
    cs2s = np.ascontiguousarray(
        np.concatenate([-sin, sin, -sin, sin], axis=0)
    )  # [128, T]
    return cs1, cs2s


def _build():
    nc = bacc.Bacc("TRN2", target_bir_lowering=False, debug=False, num_devices=NCORES)
    # host pre-arranged layouts (see kernel() below):
    #   xp:  [TC*128, DC*TCH]  chunk tci rows tci*128..: [p][dc*TCH+t']
    #   wq/wk/wv: [128, DC*HPC*DH]   [p][dc*256+n]
    #   wo:  [128, 2*D]              [p][p2*D+n]
    xp_d = nc.dram_tensor("xp", [TC * 128, DC * TCH], BF16, kind="ExternalInput")
    # wq/wk stored as [128, 2(p-half), DC, 128] so each half can be DMA'd
    # separately (the first attention unit only needs half p=0)
    wq_d = nc.dram_tensor("wq", [128, 2 * DC * 128], BF16, kind="ExternalInput")
    wk_d = nc.dram_tensor("wk", [128, 2 * DC * 128], BF16, kind="ExternalInput")
    wv_d = nc.dram_tensor("wv", [128, DC * HPC * DH], BF16, kind="ExternalInput")
    wo_d = nc.dram_tensor("wo", [128, 2 * D], BF16, kind="ExternalInput")
    y_d = nc.dram_tensor("y", [T, D], BF16, kind="ExternalOutput")

    cs1_np, cs2_np = _rope_tables()
    cs1_d = nc.inline_tensor(cs1_np, "cs1")  # [128, T]
    cs2_d = nc.inline_tensor(cs2_np, "cs2")  # [128, T]

    # causal masks (keep = 1.0): maskt for the kt0 diagonal 128-block,
    # maskd = [zeros(128) | tri(128)] for the extended kt1 diagonal block
    maskt_np = (np.arange(128)[None, :] >= np.arange(128)[:, None])
    maskd_np = (np.arange(256)[None, :] >= np.arange(128)[:, None] + 128)
    maskt_d = nc.inline_tensor(
        np.ascontiguousarray(maskt_np.astype(ml_dtypes.bfloat16)), "maskt"
    )
    maskd_d = nc.inline_tensor(
        np.ascontiguousarray(maskd_np.astype(ml_dtypes.bfloat16)), "maskd"
    )
    # 32-block-swap permutation matrix (symmetric): rope's rotate-half is
    # done as one extra matmul instead of cross-partition DMAs
    perm_np = np.zeros((128, 128), dtype=np.float32)
    sigma = (np.arange(128) + 32) % 64 + (np.arange(128) // 64) * 64
    perm_np[np.arange(128), sigma] = 1.0
    perm_d = nc.inline_tensor(
        np.ascontiguousarray(perm_np.astype(ml_dtypes.bfloat16)), "perm"
    )
    EXP = mybir.ActivationFunctionType.Exp
    COPYF = mybir.ActivationFunctionType.Copy

    import contextlib
    with tile.TileContext(nc) as tc:
        with (
            contextlib.ExitStack() as _ctx,
            tc.tile_pool(name="sb", bufs=1) as sb,
            tc.tile_pool(name="ropep", bufs=2) as ropep,
            tc.tile_pool(name="ptp", bufs=6) as ptp,
            tc.tile_pool(name="recp", bufs=2) as recp,
            tc.tile_pool(name="ysbp", bufs=3) as ysbp,
        ):
            wq = sb.tile([128, 2, DC, 128], BF16)
            wk = sb.tile([128, 2, DC, 128], BF16)
            wv = sb.tile([128, DC, HPC * DH], BF16)
            wo = sb.tile([128, 2, D], BF16)
            cs1 = sb.tile([128, T], BF16)
            cs2 = sb.tile([128, T], BF16)
            qt = [sb.tile([128, T], BF16, tag=f"qt{p}", name=f"qt{p}") for p in range(2)]
            ktt = [sb.tile([128, T], BF16, tag=f"kt{p}", name=f"kt{p}") for p in range(2)]
            maskt = sb.tile([128, 128], BF16, name="maskt")
            maskd = sb.tile([128, 256], BF16, name="maskd")
            perm = sb.tile([128, 128], BF16, name="perm")
            # v augmented with a 64-wide block of ones: the AV matmul then
            # replicates the softmax denominator across PSUM partitions
            # 64..127, so normalize needs no broadcast matmul or den DMA
            vaug = sb.tile([128, NKT // 2, 2, HPC, 2 * DH], BF16, name="vaug")
            ot = [sb.tile([128, T], BF16, tag=f"ot{p}", name=f"ot{p}") for p in range(2)]
            xts = [
                sb.tile([128, DC, TCH], BF16, tag=f"xt{tci}", name=f"xt{tci}")
                for tci in range(TC)
            ]

            # ---------------- startup DMAs ----------------
            # DMA rings stall their engine after ~6 outstanding issues (ring
            # credits), so each ring gets at most ~5 upfront; the rest are
            # issued from filler thunks once earlier transfers have drained.
            # Needed-first order: chunk-0 x split across sync+gpsimd, qk
            # weight halves on scalar, rope tables on gpsimd.
            xp_r = xp_d.ap().rearrange("(c p) (d t) -> p c d t", p=128, d=DC)
            wq_r = wq_d.ap().rearrange("p (h c n) -> p h c n", h=2, c=DC)
            wk_r = wk_d.ap().rearrange("p (h c n) -> p h c n", h=2, c=DC)
            nc.sync.dma_start(xts[0][:, 0:4], xp_r[:, 0, 0:4])
            nc.sync.dma_start(perm[:], perm_d.ap())
            nc.sync.dma_start(maskt[:], maskt_d.ap())
            nc.sync.dma_start(maskd[:], maskd_d.ap())
            nc.sync.dma_start(xts[1][:], xp_r[:, 1])
            nc.scalar.dma_start(wq[:, 0], wq_r[:, 0])
            nc.scalar.dma_start(wk[:, 0], wk_r[:, 0])
            nc.scalar.dma_start(wq[:, 1], wq_r[:, 1])
            nc.scalar.dma_start(wk[:, 1], wk_r[:, 1])
            nc.scalar.dma_start(wv[:], wv_d.ap().rearrange("p (c n) -> p c n", c=DC))
            nc.gpsimd.dma_start(xts[0][:, 4:8], xp_r[:, 0, 4:8])
            nc.gpsimd.dma_start(cs1[:], cs1_d.ap())
            nc.gpsimd.dma_start(cs2[:], cs2_d.ap())

            def load_xp2_wo():
                nc.scalar.dma_start(xts[2][:], xp_r[:, 2])
                nc.scalar.dma_start(wo[:], wo_d.ap().rearrange("p (a n) -> p a n", a=2))

            def load_xp3():
                nc.scalar.dma_start(xts[3][:], xp_r[:, 3])

            # big strided memset rides the otherwise-idle gpsimd engine so the
            # DVE (whose first op gates the PE warmup) stays clear
            nc.gpsimd.memset(vaug[:, :, :, :, DH:], 1.0)

            # PSUM: pjps(2) + sps(2 tags x 2 banks) + ops(2 tags x 1) = 8 banks
            pjps = _ctx.enter_context(tc.tile_pool(name="pjps", bufs=2, space="PSUM"))
            sps = _ctx.enter_context(tc.tile_pool(name="sps", bufs=1, space="PSUM"))
            ops = _ctx.enter_context(tc.tile_pool(name="ops", bufs=1, space="PSUM"))

            # PE warm-up: covers the startup DMA latency; ramps the PE p-state
            warm = sb.tile([128, TCH], BF16, name="warm")
            nc.vector.memset(warm, 0.0)
            wps = pjps.tile([128, TCH], F32, tag="pj", name="wps")
            for wi in range(9):
                nc.tensor.matmul(
                    wps[:], warm[:, 0:128], warm[:],
                    start=(wi == 0), stop=(wi == 8),
                )

            # ---------------- projection pieces ----------------
            def gen_proj_pieces(tci):
                """8 pieces: [q p0, q p1, k p0, k p1, v0..v3] (each ~8 MMs)."""
                tsl = slice(tci * TCH, (tci + 1) * TCH)
                xt = xts[tci]

                def qk_piece(w_sb, dest, p, pi):
                    is_q = dest is qt

                    def run():
                        ps = pjps.tile([128, TCH], F32, tag="pj", name=f"pj{tci}_{pi}")
                        for dc in range(DC):
                            nc.tensor.matmul(
                                ps[:],
                                w_sb[:, p, dc, :],
                                xt[:, dc, :],
                                start=(dc == 0),
                                stop=(dc == DC - 1),
                            )
                        # rope: scalar evacuates the psum to bf16; the
                        # 32-block swap is one matmul against the constant
                        # permutation (cs2 table pre-swapped to match), so the
                        # rope chain has no cross-partition DMAs at all
                        qkbf = ropep.tile([128, TCH], BF16, tag="qkbf", name=f"qb{tci}_{pi}")
                        nc.scalar.activation(qkbf[:], ps[:], COPYF, bias=0.0, scale=1.0)
                        ps2 = pjps.tile([128, TCH], F32, tag="pj", name=f"pw{tci}_{pi}")
                        nc.tensor.matmul(ps2[:], perm[:], qkbf[:], start=True, stop=True)
                        t1 = ropep.tile([128, TCH], BF16, tag="t1", name=f"t1_{tci}_{pi}")
                        t2 = ropep.tile([128, TCH], BF16, tag="t2", name=f"t2_{tci}_{pi}")
                        nc.vector.tensor_mul(t1[:], qkbf[:], cs1[:, tsl])
                        nc.vector.tensor_mul(t2[:], ps2[:], cs2[:, tsl])
                        nc.vector.tensor_add(dest[p][:, tsl], t1[:], t2[:])
                    return run

                def v_piece(tt):
                    def run():
                        gt = tci * 4 + tt
                        ps = pjps.tile([128, TCH], F32, tag="pj", name=f"pjv{gt}")
                        for dc in range(DC):
                            nc.tensor.matmul(
                                ps[:, : HPC * DH],
                                xt[:, dc, tt * 128 : (tt + 1) * 128],
                                wv[:, dc, :],
                                start=(dc == 0),
                                stop=(dc == DC - 1),
                            )
                        nc.vector.tensor_copy(
                            vaug[:, gt // 2, gt % 2, :, 0:DH],
                            ps[:, : HPC * DH].rearrange("p (h d) -> p h d", h=HPC),
                        )
                    return run

                pieces = []
                pi = 0
                for w_sb, dest in ((wq, qt), (wk, ktt)):
                    for p in range(2):
                        pieces.append(qk_piece(w_sb, dest, p, pi))
                        pi += 1
                for tt in range(4):
                    pieces.append(v_piece(tt))
                return pieces

            # ---------------- out-projection pieces ----------------
            YRINGS = [nc.sync, nc.gpsimd, nc.sync, nc.gpsimd]

            def gen_outproj_pieces(qc):
                def piece(tt):
                    def run():
                        gtt = qc * 4 + tt
                        ysb = ysbp.tile([128, 2, TCH], BF16, tag="ysb", name=f"ys{gtt}")
                        for ni in range(2):
                            ypsum = pjps.tile(
                                [128, TCH], F32, tag="pj", name=f"y{gtt}_{ni}"
                            )
                            for p2 in range(2):
                                nc.tensor.matmul(
                                    ypsum[:],
                                    ot[p2][:, gtt * 128 : (gtt + 1) * 128],
                                    wo[:, p2, ni * TCH : (ni + 1) * TCH],
                                    start=(p2 == 0),
                                    stop=(p2 == 1),
                                )
                            nc.vector.tensor_copy(ysb[:, ni, :], ypsum[:])
                        YRINGS[tt].dma_start(
                            y_d.ap()[gtt * 128 : (gtt + 1) * 128, :],
                            ysb.rearrange("p a b -> p (a b)"),
                        )
                    return run
                return [piece(tt) for tt in range(4)]

            # ---------------- attention: head-pair units ----------------
            pts = {}     # (qc,hp) -> {g: [pt_par0, pt_par1]}
            opsums = {}  # (qc,hp) -> [psum_par0, psum_par1]

            def emit_s_group(qc, hp, g):
                """Row-tiled S pair + exp + mask for k-tile pair (2g, 2g+1)."""
                kt0 = 2 * g
                off0 = max(0, 128 * kt0 - TCH * qc)
                qsl = slice(qc * TCH, (qc + 1) * TCH)
                spts, ptts = [], []
                for par in (0, 1):
                    spt = sps.tile(
                        [128, 2, TCH], F32, tag=f"s{par}", name=f"s{qc}_{hp}_{g}_{par}"
                    )
                    pt = ptp.tile(
                        [128, 2, TCH], BF16, tag=f"pt{par}", name=f"p{qc}_{hp}_{g}_{par}"
                    )
                    spts.append(spt)
                    ptts.append(pt)
                pts[(qc, hp)][g] = ptts
                for j in (0, 1):
                    kt = kt0 + j
                    for par in (0, 1):
                        nc.tensor.matmul(
                            spts[par][:, j, off0:],
                            ktt[hp][par * 64 : (par + 1) * 64, kt * 128 : (kt + 1) * 128],
                            qt[hp][par * 64 : (par + 1) * 64, qsl][:, off0:],
                            start=True,
                            stop=True,
                        )
                for par in (0, 1):
                    if off0 == 0:
                        nc.scalar.activation(
                            ptts[par].rearrange("p a b -> p (a b)"),
                            spts[par].rearrange("p a b -> p (a b)"),
                            EXP, bias=0.0, scale=0.125,
                        )
                    else:
                        nc.scalar.activation(
                            ptts[par][:, :, off0:], spts[par][:, :, off0:],
                            EXP, bias=0.0, scale=0.125,
                        )
                if kt0 >= 4 * qc:  # diagonal pair
                    for par in (0, 1):
                        nc.vector.tensor_mul(
                            ptts[par][:, 0, off0 : off0 + 128],
                            ptts[par][:, 0, off0 : off0 + 128],
                            maskt[:],
                        )
                        nc.vector.tensor_mul(
                            ptts[par][:, 1, off0 : off0 + 256],
                            ptts[par][:, 1, off0 : off0 + 256],
                            maskd[:],
                        )

            def emit_av_group(qc, hp, g):
                nkt = 4 * qc + 4
                kt0 = 2 * g
                ptts = pts[(qc, hp)].pop(g)
                if g == 0:
                    opsums[(qc, hp)] = ops.tile(
                        [128, 2, TCH], F32, tag="o", name=f"o{qc}_{hp}"
                    )
                for par in (0, 1):
                    h = 2 * hp + par
                    for j in (0, 1):
                        kt = kt0 + j
                        off = max(0, 128 * kt - TCH * qc)
                        nc.tensor.matmul(
                            opsums[(qc, hp)][:, par, off:],
                            vaug[:, g, j, h, :],
                            ptts[par][:, j, off:],
                            start=(kt == 0),
                            stop=(kt == nkt - 1),
                        )

            def emit_evac_norm(qc, hp):
                """Normalize both heads straight out of PSUM: partitions
                64-127 of each AV psum hold the softmax denominator (vaug's
                ones block), so a reciprocal + one multiply per head does it.
                ot layout par-swapped: partitions 0-63 = odd head, 64-127 =
                even head."""
                qsl = slice(qc * TCH, (qc + 1) * TCH)
                opsum = opsums[(qc, hp)]
                den = recp.tile([64, 2, TCH], F32, tag="dn", name=f"dn{qc}_{hp}")
                # custom DVE ops can't source PSUM: bounce den via SBUF
                nc.vector.tensor_copy(den[:], opsum[DH : 2 * DH, :, :])
                rec = recp.tile([64, 2, TCH], F32, tag="rc", name=f"rc{qc}_{hp}")
                nc.vector.reciprocal_approx_fast(
                    out=rec.rearrange("p a b -> p (a b)"),
                    in_=den.rearrange("p a b -> p (a b)"),
                )
                nc.vector.tensor_mul(
                    ot[hp][0:64, qsl], opsum[0:DH, 1, :], rec[:, 1, :]
                )
                nc.vector.tensor_mul(
                    ot[hp][64:128, qsl], opsum[0:DH, 0, :], rec[:, 0, :]
                )

            # ---------------- emission schedule ----------------
            # proj(0) upfront; the two pieces the first attention unit needs
            # (q p0, k p0) go first
            p0 = gen_proj_pieces(0)
            for f in [p0[0], p0[2], p0[1], p0[3]] + p0[4:]:
                f()

            p1 = gen_proj_pieces(1)
            p2 = gen_proj_pieces(2)
            p3 = gen_proj_pieces(3)

            op0 = gen_outproj_pieces(0)
            op1 = gen_outproj_pieces(1)
            op2 = gen_outproj_pieces(2)

            units = [(qc, hp) for qc in range(TC) for hp in range(2)]
            # fillers per unit (consumed one per S-group slot; leftovers run
            # at the unit boundary). Invocation order must respect the norms
            # each outproj chunk reads.
            unit_fillers = {
                0: [load_xp2_wo, load_xp3],
                1: [p1[0], p1[2], p1[1], p1[3]],      # q1/k1 (2 slots + spill)
                2: p1[4:8],                           # v(1)
                3: [p2[0], p2[2], p2[1], p2[3]] + op0,
                4: p2[4:8] + [p3[0], p3[2]],
                5: [p3[1], p3[3]] + op1,
                6: p3[4:8],                           # v(3)
                7: op2[0:2],  # op2[2:4] held back to cover the drain norm
            }

            # flat S-slot stream with a fixed AV lag: the AV matmuls for
            # S slot i run at slot i+LAG, so the final unit's AV doesn't
            # bunch up in the drain
            LAG = 4
            slots = []
            for ui, (qc, hp) in enumerate(units):
                for g in range((4 * qc + 4) // 2):
                    slots.append((ui, qc, hp, g))

            def do_av_slot(j):
                ui, qc, hp, g = slots[j]
                emit_av_group(qc, hp, g)
                if g == (4 * qc + 4) // 2 - 1:  # unit's last AV group
                    emit_evac_norm(qc, hp)

            fill = []
            for i, (ui, qc, hp, g) in enumerate(slots):
                if g == 0:
                    fill = list(unit_fillers.get(ui, []))
                    pts[(qc, hp)] = {}
                    opsums[(qc, hp)] = None
                if fill:
                    fill.pop(0)()
                if i >= LAG:
                    do_av_slot(i - LAG)
                emit_s_group(qc, hp, g)
                if g == (4 * qc + 4) // 2 - 1:
                    for f in fill:
                        f()
                    fill = []

            # drain: the last LAG AV slots, final norm, outproj of last chunk.
            # The outproj is emitted ni-granular with both p2=0 matmuls (which
            # depend only on norm(3,0)) ahead of the p2=1 ones, so the PE has
            # work while the final norm's DVE chain runs; casts ride the
            # now-idle scalar engine.
            for j in range(len(slots) - LAG, len(slots)):
                do_av_slot(j)
            op2[2]()
            op2[3]()
            # last chunk's outproj: the freed S psum banks hold 2-bank ypsums,
            # p2=0 matmuls (independent of the final norm) lead, one merged
            # scalar cast per piece
            for tt in range(4):
                gtt = 3 * 4 + tt
                ypsum = sps.tile(
                    [128, 2, TCH], F32, tag=f"s{tt % 2}", name=f"y{gtt}"
                )
                for p2 in range(2):
                    for ni in range(2):
                        nc.tensor.matmul(
                            ypsum[:, ni, :],
                            ot[p2][:, gtt * 128 : (gtt + 1) * 128],
                            wo[:, p2, ni * TCH : (ni + 1) * TCH],
                            start=(p2 == 0), stop=(p2 == 1),
                        )
                ysb = ysbp.tile([128, 2, TCH], BF16, tag="ysb", name=f"ys{gtt}")
                nc.scalar.activation(
                    ysb.rearrange("p a b -> p (a b)"),
                    ypsum.rearrange("p a b -> p (a b)"),
                    COPYF, bias=0.0, scale=1.0,
                )
                YRINGS[tt].dma_start(
                    y_d.ap()[gtt * 128 : (gtt + 1) * 128, :],
                    ysb.rearrange("p a b -> p (a b)"),
                )
    nc.compile()
    return nc


_NC_CACHE = []


def _get_nc():
    if not _NC_CACHE:
        _NC_CACHE.append(_build())
    return _NC_CACHE[0]


_LAST_RESULTS = []  # stashed BassKernelResults for test harness introspection


def _wo_rows_parswap(Wout_rows):
    """Reorder the 256 Wout rows so each 128-row pair block is [odd-head 64 | even-head 64]."""
    out = np.empty_like(Wout_rows)
    for hp in range(2):
        blk = Wout_rows[hp * 128 : (hp + 1) * 128]
        out[hp * 128 : hp * 128 + 64] = blk[64:128]
        out[hp * 128 + 64 : (hp + 1) * 128] = blk[0:64]
    return out


def kernel(x, Wqkv, Wout, _trace=False, **_trace_kwargs):
    x = np.asarray(x, dtype=np.float32)
    Wqkv = np.asarray(Wqkv, dtype=np.float32)
    Wout = np.asarray(Wout, dtype=np.float32)

    nc = _get_nc()
    bf = ml_dtypes.bfloat16
    in_maps = []
    for c in range(NCORES):
        b, g = divmod(c, HPC)
        cols = slice(g * HPC * DH, (g + 1) * HPC * DH)
        rows = slice(g * HPC * DH, (g + 1) * HPC * DH)
        xT = x[b].T.astype(bf)  # [D, T]
        xp = np.ascontiguousarray(
            xT.reshape(DC, 128, TC, TCH).transpose(2, 1, 0, 3).reshape(TC * 128, DC * TCH)
        )

        def wprep(w):  # [D, 256] -> [128, DC*256]
            return np.ascontiguousarray(
                w.astype(bf).reshape(DC, 128, HPC * DH).transpose(1, 0, 2).reshape(128, -1)
            )

        def wprep_qk(w):  # [D, 256] -> [128, 2*DC*128] ([p][half][dc][n])
            return np.ascontiguousarray(
                w.astype(bf).reshape(DC, 128, 2, 128).transpose(1, 2, 0, 3).reshape(128, -1)
            )

        wo_ = np.ascontiguousarray(
            _wo_rows_parswap(Wout[rows, :]).astype(bf)
            .reshape(2, 128, D).transpose(1, 0, 2).reshape(128, 2 * D)
        )
        in_maps.append(
            {
                "xp": xp,
                "wq": wprep_qk(Wqkv[:, 0:D][:, cols]),
                "wk": wprep_qk(Wqkv[:, D : 2 * D][:, cols]),
                "wv": wprep(Wqkv[:, 2 * D : 3 * D][:, cols]),
                "wo": wo_,
            }
        )

    res = run_bass_kernel_spmd(
        nc, in_maps, core_ids=list(range(NCORES)), trace=_trace, **_trace_kwargs
    )
    _LAST_RESULTS.clear()
    _LAST_RESULTS.append(res)

    out = np.zeros((B, T, D), dtype=np.float32)
    for c in range(NCORES):
        b = c // HPC
        out[b] += res.results[c]["y"].astype(np.float32)
    return out
